# revision 1
# baseline (speedup 1.0000x reference)
"""ConformerBlock Trainium2 kernel.

Data-parallel over batch: B=8 = one batch element per NeuronCore, no
collectives (every module in the block is per-sample, including the
GroupNorm which normalizes over (C,T) of each sample).

Per-core layout strategy:
  - residual `y` kept time-major [T=1024, D=512] as [128, 8, 512] SBUF tile
  - LayerNorm stats via bn_stats per 128-row time tile; the LN gamma/beta
    are folded into the *following* matmul's weights on the host, so on
    chip LN is just (x - m) * rsqrt(v+eps) (one scalar-engine op)
  - the normalized tile is PE-transposed to feature-major [D, T] which
    feeds every matmul (weights stationary [K,M], activations moving
    [K,N<=512]); all matmuls run in float32r (FP22 multiply, FP32
    accumulate, full PE rate at N>=256)
  - windowed attention (|i-j|<=32): per 256-query pair, a 384-wide key
    window; scores + additive mask + exp (no max-sub: scores are O(1)),
    row-normalize, PE-transpose the probs, then AV with time-major V
  - depthwise conv K=31 runs on the PE as 31 accumulating matmuls with
    per-tap diagonal weight matrices (prebuilt on host, streamed from
    DRAM) against sliding windows of the zero-padded GLU output
"""

import numpy as np

import concourse.bass as bass
import concourse.bacc as bacc
import concourse.tile as tile
from concourse import mybir
from concourse.bass_utils import run_bass_kernel_spmd
from concourse import bass_isa

F32 = mybir.dt.float32
F32R = mybir.dt.float32r
AF = mybir.ActivationFunctionType
OP = mybir.AluOpType
AX = mybir.AxisListType

B, T, D, H, KTAP, WIN = 8, 1024, 512, 8, 31, 64
DF = 4 * D            # 2048 ffn hidden
DC = 2 * D            # 1024 conv channels
DH = D // H           # 64
EPS = 1e-5
P = 128
TT_N = T // P         # 8 time tiles
D_T = D // P          # 4
DF_T = DF // P        # 16
DC_T = DC // P        # 8
PAD = 32              # ln_t / kpad leading pad
KP_W = PAD + T + 96   # 1152 padded time width (feature-major)
CPAD = 15             # conv halo
G_W = T + 2 * CPAD    # 1054
NEG = -30000.0

N_CORES = 8


def _dram_vec_bcast_ap(dram_ap, n):
    """AP reading a [n] dram vector broadcast across 128 partitions."""
    return bass.AP(tensor=dram_ap.tensor, offset=dram_ap.offset,
                   ap=[[0, P], [1, n]])


def _build(nc):
    dp = nc.declare_dram_parameter
    x_d = dp("x", [T, D], F32, isOutput=False)
    w1f_d = dp("w1f", [D, DF], F32R, isOutput=False)
    b1f_d = dp("b1f", [P, DF_T], F32, isOutput=False)
    w2_d = dp("w2", [DF, D], F32R, isOutput=False)
    b2h_d = dp("b2h", [D], F32, isOutput=False)
    qkvw_d = dp("qkvw", [D, 3 * D], F32R, isOutput=False)
    qb_d = dp("qb", [P, 4], F32, isOutput=False)
    kb_d = dp("kb", [P, 4], F32, isOutput=False)
    vb_d = dp("vb", [64, 8], F32, isOutput=False)
    outw_d = dp("outw", [D, D], F32R, isOutput=False)
    outb_d = dp("outb", [D], F32, isOutput=False)
    pw1t_d = dp("pw1t", [D, 2 * DC], F32R, isOutput=False)
    ba1_d = dp("ba1", [P, DC_T], F32, isOutput=False)
    ba2_d = dp("ba2", [P, DC_T], F32, isOutput=False)
    dwdiag_d = dp("dwdiag", [DC_T, KTAP, P, P], F32R, isOutput=False)
    dwb_d = dp("dwb", [P, DC_T], F32, isOutput=False)
    gnw_d = dp("gnw", [P, DC_T], F32, isOutput=False)
    gnb_d = dp("gnb", [P, DC_T], F32, isOutput=False)
    pw2t_d = dp("pw2t", [DC, D], F32R, isOutput=False)
    pw2b_d = dp("pw2b", [D], F32, isOutput=False)
    w1f2_d = dp("w1f2", [D, DF], F32R, isOutput=False)
    b1f2_d = dp("b1f2", [P, DF_T], F32, isOutput=False)
    w22_d = dp("w22", [DF, D], F32R, isOutput=False)
    b2h2_d = dp("b2h2", [D], F32, isOutput=False)
    flnw_d = dp("flnw", [D], F32, isOutput=False)
    flnb_d = dp("flnb", [D], F32, isOutput=False)
    ident_d = dp("ident", [P, P], F32R, isOutput=False)
    zeros_d = dp("zeros", [128], F32R, isOutput=False)
    masks_d = dp("masks", [4, P, 384], F32, isOutput=False)
    out_d = dp("y_out", [T, D], F32, isOutput=True)

    with tile.TileContext(nc) as tc:
        with (
            tc.tile_pool(name="const", bufs=1) as cpool,
            tc.tile_pool(name="resid", bufs=1) as rpool,
            tc.tile_pool(name="lnt", bufs=1) as lpool,
            tc.tile_pool(name="big", bufs=1) as bigp,
            tc.tile_pool(name="w2res", bufs=1) as w2p,
            tc.tile_pool(name="wbig", bufs=1) as wbp,
            tc.tile_pool(name="med", bufs=1) as medp,
            tc.tile_pool(name="wd", bufs=2) as wdp,
            tc.tile_pool(name="small", bufs=4) as smp,
            tc.tile_pool(name="smx", bufs=4) as smxp,
            tc.tile_pool(name="attnT", bufs=2) as atp,
            tc.tile_pool(name="psA", bufs=2, space="PSUM") as psA,
            tc.tile_pool(name="psB", bufs=4, space="PSUM") as psB,
            tc.tile_pool(name="psC", bufs=2, space="PSUM") as psC,
        ):
            # ---------------- constants ----------------
            ident = cpool.tile([P, P], F32R, tag="ident")
            nc.sync.dma_start(out=ident, in_=ident_d[:, :])
            identR = ident
            eps_t = cpool.tile([P, 1], F32, tag="eps")
            nc.vector.memset(eps_t, EPS)

            def bcast_tile(dram_ap, tag):
                t_ = cpool.tile([P, D], F32, tag=tag)
                nc.sync.dma_start(out=t_, in_=_dram_vec_bcast_ap(dram_ap, D))
                return t_

            b2h_bc = bcast_tile(b2h_d[:], "b2h")
            outb_bc = bcast_tile(outb_d[:], "outb")
            pw2b_bc = bcast_tile(pw2b_d[:], "pw2b")
            b2h2_bc = bcast_tile(b2h2_d[:], "b2h2")
            flnw_bc = bcast_tile(flnw_d[:], "flnw")
            flnb_bc = bcast_tile(flnb_d[:], "flnb")

            def c2d(dram, n, tag):
                t_ = cpool.tile([P, n], F32, tag=tag)
                nc.sync.dma_start(out=t_, in_=dram[:, :])
                return t_

            b1f_t = c2d(b1f_d, DF_T, "b1f")
            qb_t = c2d(qb_d, 4, "qb")
            kb_t = c2d(kb_d, 4, "kb")
            vb_t = cpool.tile([64, 8], F32, tag="vb")
            nc.sync.dma_start(out=vb_t, in_=vb_d[:, :])
            ba1_t = c2d(ba1_d, DC_T, "ba1")
            ba2_t = c2d(ba2_d, DC_T, "ba2")
            dwb_t = c2d(dwb_d, DC_T, "dwb")
            gnw_t = c2d(gnw_d, DC_T, "gnw")
            gnb_t = c2d(gnb_d, DC_T, "gnb")
            b1f2_t = c2d(b1f2_d, DF_T, "b1f2")

            masks_t = cpool.tile([P, 4, 384], F32, tag="masks")
            for i in range(4):
                nc.sync.dma_start(out=masks_t[:, i, :], in_=masks_d[i, :, :])
            # mask index: 0=mid qh0, 1=mid qh1, 2=left (qp0,qh0), 3=right (qp3,qh1)

            zeros_ap = zeros_d[:]

            def zfill(out_ap, n1, n2):
                nc.sync.dma_start(out=out_ap, in_=bass.AP(
                    tensor=zeros_ap.tensor, offset=zeros_ap.offset,
                    ap=[[0, P], [0, n1], [1, n2]]))

            # ---------------- residual load ----------------
            y = rpool.tile([P, TT_N, D], F32, tag="y")
            nc.sync.dma_start(out=y,
                              in_=x_d.rearrange("(a p) d -> p a d", p=P))

            # ---------------- layernorm -> feature-major ----------------
            def layer_norm_t(need_pad):
                """LN of y (no gamma/beta: folded into next weights), transposed
                into a fresh feature-major [128, D_T, KP_W] tile (data at col
                PAD..PAD+T)."""
                ln_t = lpool.tile([P, D_T, KP_W], F32R, tag="lnt")
                if need_pad:
                    zfill(ln_t[:, :, 0:PAD], D_T, PAD)
                    zfill(ln_t[:, :, PAD + T:KP_W], D_T, KP_W - PAD - T)
                for tt in range(TT_N):
                    mv = smp.tile([P, 2], F32, tag="mv")
                    st6 = smp.tile([P, 6], F32, tag="st6")
                    nc.vector.bn_stats(out=st6, in_=y[:, tt, :])
                    nc.vector.bn_aggr(out=mv, in_=st6)
                    r_ = smp.tile([P, 1], F32, tag="r")
                    nc.scalar.activation(out=r_, in_=mv[:, 1:2], func=AF.Sqrt,
                                         bias=eps_t, scale=1.0)
                    nc.vector.reciprocal(out=r_, in_=r_)
                    nmr = smp.tile([P, 1], F32, tag="nmr")
                    nc.vector.tensor_scalar(out=nmr, in0=mv[:, 0:1], scalar1=r_,
                                            scalar2=-1.0, op0=OP.mult, op1=OP.mult)
                    lnp = smp.tile([P, D], F32R, tag="lnp", bufs=2)
                    nc.vector.tensor_scalar(out=lnp, in0=y[:, tt, :], scalar1=r_,
                                            scalar2=nmr, op0=OP.mult, op1=OP.add)
                    tp = psB.tile([P, D], F32R, tag="tp")
                    for dt in range(D_T):
                        nc.tensor.transpose(tp[:, dt * P:(dt + 1) * P],
                                            lnp[:, dt * P:(dt + 1) * P],
                                            identR)
                        nc.scalar.activation(
                            out=ln_t[:, dt, PAD + tt * P:PAD + (tt + 1) * P],
                            in_=tp[:, dt * P:(dt + 1) * P],
                            func=AF.Copy)
                return ln_t

            # ---------------- FFN (macaron half-residual) ----------------
            def ffn(w1_dram, b1_tile, w2_dram, b2h_bcast):
                ln_t = layer_norm_t(False)
                w2r = w2p.tile([P, DF_T, D], F32R, tag="w2res")
                for kt in range(DF_T):
                    nc.sync.dma_start(out=w2r[:, kt, :],
                                      in_=w2_dram[kt * P:(kt + 1) * P, :])
                w1t = wbp.tile([P, D_T, DF], F32R, tag="wbig")
                for kt in range(D_T):
                    nc.sync.dma_start(out=w1t[:, kt, :],
                                      in_=w1_dram[kt * P:(kt + 1) * P, :])
                for th in range(2):
                    h1 = bigp.tile([P, DF_T, D], F32R, tag="big")
                    for ft in range(DF_T):
                        ps = psA.tile([P, D], F32, tag="mm")
                        for kt in range(D_T):
                            nc.tensor.matmul(
                                ps,
                                lhsT=w1t[:, kt, ft * P:(ft + 1) * P],
                                rhs=ln_t[:, kt, PAD + th * D:PAD + (th + 1) * D],
                                start=(kt == 0), stop=(kt == D_T - 1))
                        nc.scalar.activation(out=h1[:, ft, :], in_=ps, func=AF.Silu,
                                             bias=b1_tile[:, ft:ft + 1], scale=1.0)
                    for tc in range(4):
                        ps2 = psA.tile([P, D], F32, tag="mm")
                        for kt in range(DF_T):
                            nc.tensor.matmul(
                                ps2,
                                lhsT=h1[:, kt, tc * P:(tc + 1) * P],
                                rhs=w2r[:, kt, :],
                                start=(kt == 0), stop=(kt == DF_T - 1))
                        g_tc = th * 4 + tc
                        nc.vector.scalar_tensor_tensor(
                            out=y[:, g_tc, :], in0=ps2, scalar=0.5,
                            in1=y[:, g_tc, :], op0=OP.mult, op1=OP.add)
                        nc.vector.tensor_add(out=y[:, g_tc, :], in0=y[:, g_tc, :],
                                             in1=b2h_bcast)

            # ================= FFN1 =================
            ffn(w1f_d, b1f_t, w2_d, b2h_bc)

            # ================= attention =================
            ln_t = layer_norm_t(True)
            qkvt = wbp.tile([P, D_T, 3 * D], F32R, tag="wbig")
            for kt in range(D_T):
                nc.sync.dma_start(out=qkvt[:, kt, :],
                                  in_=qkvw_d[kt * P:(kt + 1) * P, :])
            qk = bigp.tile([P, 4 * T + 4 * KP_W], F32R, tag="big")
            q_all = qk[:, 0:4 * T].rearrange("p (h t) -> p h t", h=4)
            kpad = qk[:, 4 * T:4 * T + 4 * KP_W].rearrange("p (h t) -> p h t", h=4)
            zfill(kpad[:, :, 0:PAD], 4, PAD)
            zfill(kpad[:, :, PAD + T:KP_W], 4, KP_W - PAD - T)
            # q, k: feature-major [head-pair 128, T]
            for hp in range(4):
                for tn in range(2):
                    psq = psA.tile([P, D], F32, tag="mm")
                    for kt in range(D_T):
                        nc.tensor.matmul(
                            psq,
                            lhsT=qkvt[:, kt, hp * P:(hp + 1) * P],
                            rhs=ln_t[:, kt, PAD + tn * D:PAD + (tn + 1) * D],
                            start=(kt == 0), stop=(kt == D_T - 1))
                    nc.scalar.activation(out=q_all[:, hp, tn * D:(tn + 1) * D],
                                         in_=psq, func=AF.Identity,
                                         bias=qb_t[:, hp:hp + 1], scale=1.0)
                    psk = psA.tile([P, D], F32, tag="mm")
                    for kt in range(D_T):
                        nc.tensor.matmul(
                            psk,
                            lhsT=qkvt[:, kt, D + hp * P:D + (hp + 1) * P],
                            rhs=ln_t[:, kt, PAD + tn * D:PAD + (tn + 1) * D],
                            start=(kt == 0), stop=(kt == D_T - 1))
                    nc.scalar.activation(
                        out=kpad[:, hp, PAD + tn * D:PAD + (tn + 1) * D],
                        in_=psk, func=AF.Identity,
                        bias=kb_t[:, hp:hp + 1], scale=1.0)
            # v: time-major, stored at +32 row offset (9 slots of 128)
            vpad = w2p.tile([P, 9, D], F32R, tag="w2res")
            for vt in range(9):
                psv = psA.tile([P, D], F32, tag="mm")
                for kt in range(D_T):
                    nc.tensor.matmul(
                        psv,
                        lhsT=ln_t[:, kt, vt * P:(vt + 1) * P],
                        rhs=qkvt[:, kt, 2 * D:3 * D],
                        start=(kt == 0), stop=(kt == D_T - 1))
                nc.scalar.activation(out=vpad[:, vt, :], in_=psv, func=AF.Copy)

            o_t = [medp.tile([64, T], F32R, tag=f"med{i}", name=f"o_t{i}")
                   for i in range(8)]
            for hp in range(4):
                for qp in range(4):
                    for hi in range(2):
                        h = 2 * hp + hi
                        base = hi * 64
                        av = psB.tile([64, 256], F32, tag="tp", name=f"av{h}_{qp}")
                        smx_h = []
                        for qh in range(2):
                            sc = psC.tile([P, 384], F32, tag="sc")
                            nc.tensor.matmul(
                                sc,
                                lhsT=q_all[base:base + 64, hp,
                                           qp * 256 + qh * P:qp * 256 + (qh + 1) * P],
                                rhs=kpad[base:base + 64, hp,
                                         qp * 256:qp * 256 + 384],
                                start=True, stop=True)
                            if qh == 0:
                                mi = 2 if qp == 0 else 0
                            else:
                                mi = 3 if qp == 3 else 1
                            smx = smxp.tile([P, 384], F32R, tag="smx")
                            nc.vector.tensor_add(out=smx, in0=sc,
                                                 in1=masks_t[:, mi, :])
                            lsum = smp.tile([P, 1], F32, tag="lsum")
                            nc.scalar.activation(out=smx, in_=smx.bitcast(F32), func=AF.Exp,
                                                 accum_out=lsum)
                            rr = smp.tile([P, 1], F32, tag="rr")
                            nc.vector.reciprocal(out=rr, in_=lsum)
                            nc.vector.tensor_scalar(out=smx, in0=smx.bitcast(F32), scalar1=rr,
                                                    scalar2=None, op0=OP.mult)
                            smx_h.append(smx)
                        for sb in range(3):
                            tp2 = psB.tile([P, 256], F32R, tag="tp")
                            for qh in range(2):
                                nc.tensor.transpose(
                                    tp2[:, qh * P:(qh + 1) * P],
                                    smx_h[qh][:, sb * P:(sb + 1) * P],
                                    identR)
                            at = atp.tile([P, 256], F32R, tag="at")
                            nc.scalar.activation(out=at, in_=tp2, func=AF.Copy)
                            vt = qp * 2 + sb
                            nc.tensor.matmul(
                                av,
                                lhsT=vpad[:, vt, h * DH:(h + 1) * DH],
                                rhs=at,
                                start=(sb == 0), stop=(sb == 2))
                        nc.scalar.activation(
                            out=o_t[h][:, qp * 256:(qp + 1) * 256],
                            in_=av, func=AF.Identity,
                            bias=vb_t[:, h:h + 1], scale=1.0)
            # out projection + residual
            outwt = wbp.tile([64, 8, D], F32R, tag="wbig")
            nc.sync.dma_start(out=outwt,
                              in_=outw_d.rearrange("(a p) d -> p a d", p=64))
            for tc in range(TT_N):
                pso = psA.tile([P, D], F32, tag="mm")
                for h in range(8):
                    nc.tensor.matmul(
                        pso,
                        lhsT=o_t[h][:, tc * P:(tc + 1) * P],
                        rhs=outwt[:, h, :],
                        start=(h == 0), stop=(h == 7))
                nc.vector.tensor_add(out=y[:, tc, :], in0=y[:, tc, :], in1=pso)
                nc.vector.tensor_add(out=y[:, tc, :], in0=y[:, tc, :], in1=outb_bc)

            # ================= conv module =================
            ln_t = layer_norm_t(False)
            pw1tt = wbp.tile([P, D_T, 2 * DC], F32R, tag="wbig")
            for kt in range(D_T):
                nc.sync.dma_start(out=pw1tt[:, kt, :],
                                  in_=pw1t_d[kt * P:(kt + 1) * P, :])
            g = bigp.tile([P, DC_T, G_W], F32R, tag="big")
            zfill(g[:, :, 0:CPAD], DC_T, CPAD)
            zfill(g[:, :, CPAD + T:G_W], DC_T, CPAD)
            for ct in range(DC_T):
                for tn in range(2):
                    ps_a2 = psA.tile([P, D], F32, tag="mm")
                    for kt in range(D_T):
                        nc.tensor.matmul(
                            ps_a2,
                            lhsT=pw1tt[:, kt, DC + ct * P:DC + (ct + 1) * P],
                            rhs=ln_t[:, kt, PAD + tn * D:PAD + (tn + 1) * D],
                            start=(kt == 0), stop=(kt == D_T - 1))
                    sig = smp.tile([P, D], F32, tag="sig", bufs=2)
                    nc.scalar.activation(out=sig, in_=ps_a2, func=AF.Sigmoid,
                                         bias=ba2_t[:, ct:ct + 1], scale=1.0)
                    ps_a1 = psA.tile([P, D], F32, tag="mm")
                    for kt in range(D_T):
                        nc.tensor.matmul(
                            ps_a1,
                            lhsT=pw1tt[:, kt, ct * P:(ct + 1) * P],
                            rhs=ln_t[:, kt, PAD + tn * D:PAD + (tn + 1) * D],
                            start=(kt == 0), stop=(kt == D_T - 1))
                    nc.vector.scalar_tensor_tensor(
                        out=g[:, ct, CPAD + tn * D:CPAD + (tn + 1) * D],
                        in0=ps_a1, scalar=ba1_t[:, ct:ct + 1], in1=sig,
                        op0=OP.add, op1=OP.mult)
            # depthwise conv: 31 accumulating diag matmuls, sliding rhs window
            cv = [medp.tile([P, T], F32R, tag=f"med{i}", name=f"cv{i}") for i in range(DC_T)]
            for ct in range(DC_T):
                pcs = [psA.tile([P, D], F32, tag="mm", name=f"pcs{ct}_{k}") for k in range(2)]
                for j in range(KTAP):
                    dg = wdp.tile([P, P], F32R, tag="wd")
                    nc.sync.dma_start(out=dg, in_=dwdiag_d[ct, j, :, :])
                    for tn in range(2):
                        nc.tensor.matmul(
                            pcs[tn],
                            lhsT=dg,
                            rhs=g[:, ct, j + tn * D:j + (tn + 1) * D],
                            start=(j == 0), stop=(j == KTAP - 1))
                for tn in range(2):
                    nc.scalar.activation(out=cv[ct][:, tn * D:(tn + 1) * D],
                                         in_=pcs[tn], func=AF.Identity,
                                         bias=dwb_t[:, ct:ct + 1], scale=1.0)
            # GroupNorm(1 group over C,T) stats
            stats_pk = smp.tile([P, 16], F32, tag="stpk")
            for ct in range(DC_T):
                st = smp.tile([P, 2, 6], F32, tag="st26")
                nc.vector.bn_stats(out=st[:, 0, :], in_=cv[ct][:, 0:D].bitcast(F32))
                nc.vector.bn_stats(out=st[:, 1, :], in_=cv[ct][:, D:T].bitcast(F32))
                mv = smp.tile([P, 2], F32, tag="mv")
                nc.vector.bn_aggr(out=mv, in_=st)
                nc.vector.tensor_copy(out=stats_pk[:, ct:ct + 1], in_=mv[:, 0:1])
                nc.vector.scalar_tensor_tensor(
                    out=stats_pk[:, 8 + ct:9 + ct], in0=mv[:, 0:1],
                    scalar=mv[:, 0:1], in1=mv[:, 1:2], op0=OP.mult, op1=OP.add)
            red = smp.tile([P, 16], F32, tag="gred")
            nc.gpsimd.partition_all_reduce(red, stats_pk, channels=P,
                                           reduce_op=bass_isa.ReduceOp.add)
            sums = smp.tile([P, 2], F32, tag="sums")
            nc.vector.tensor_reduce(out=sums,
                                    in_=red.rearrange("p (a b) -> p a b", a=2),
                                    axis=AX.X, op=OP.add)
            mq = smp.tile([P, 2], F32, tag="mq")  # [mu, E[x^2]] on every partition
            nc.vector.tensor_scalar(out=mq, in0=sums, scalar1=1.0 / DC,
                                    scalar2=None, op0=OP.mult)
            # var = E[x^2] - mu^2
            var_t = smp.tile([P, 1], F32, tag="var")
            nc.vector.tensor_scalar(out=var_t, in0=mq[:, 0:1], scalar1=mq[:, 0:1],
                                    scalar2=-1.0, op0=OP.mult, op1=OP.mult)
            nc.vector.tensor_add(out=var_t, in0=var_t, in1=mq[:, 1:2])
            rstd = smp.tile([P, 1], F32, tag="rstd")
            nc.scalar.activation(out=rstd, in_=var_t, func=AF.Sqrt,
                                 bias=eps_t, scale=1.0)
            nc.vector.reciprocal(out=rstd, in_=rstd)
            # per-channel-tile scale/shift + SiLU, then pw2 + residual
            pw2tt = w2p.tile([P, DC_T, D], F32R, tag="w2res")
            for kt in range(DC_T):
                nc.sync.dma_start(out=pw2tt[:, kt, :],
                                  in_=pw2t_d[kt * P:(kt + 1) * P, :])
            for ct in range(DC_T):
                s_c = smp.tile([P, 1], F32, tag="s_c")
                nc.vector.tensor_scalar(out=s_c, in0=gnw_t[:, ct:ct + 1],
                                        scalar1=rstd, scalar2=None,
                                        op0=OP.mult)
                t_c = smp.tile([P, 1], F32, tag="t_c")
                nc.vector.tensor_scalar(out=t_c, in0=s_c, scalar1=mq[:, 0:1],
                                        scalar2=-1.0, op0=OP.mult, op1=OP.mult)
                nc.vector.tensor_add(out=t_c, in0=t_c, in1=gnb_t[:, ct:ct + 1])
                nc.scalar.activation(out=cv[ct], in_=cv[ct].bitcast(F32), func=AF.Silu,
                                     bias=t_c, scale=s_c)
            for tc in range(TT_N):
                psp = psA.tile([P, D], F32, tag="mm")
                for kt in range(DC_T):
                    nc.tensor.matmul(
                        psp,
                        lhsT=cv[kt][:, tc * P:(tc + 1) * P],
                        rhs=pw2tt[:, kt, :],
                        start=(kt == 0), stop=(kt == DC_T - 1))
                nc.vector.tensor_add(out=y[:, tc, :], in0=y[:, tc, :], in1=psp)
                nc.vector.tensor_add(out=y[:, tc, :], in0=y[:, tc, :], in1=pw2b_bc)

            # ================= FFN2 =================
            ffn(w1f2_d, b1f2_t, w22_d, b2h2_bc)

            # ================= final LN + store =================
            for tt in range(TT_N):
                mv = smp.tile([P, 2], F32, tag="mv")
                st6 = smp.tile([P, 6], F32, tag="st6")
                nc.vector.bn_stats(out=st6, in_=y[:, tt, :])
                nc.vector.bn_aggr(out=mv, in_=st6)
                r_ = smp.tile([P, 1], F32, tag="r")
                nc.scalar.activation(out=r_, in_=mv[:, 1:2], func=AF.Sqrt,
                                     bias=eps_t, scale=1.0)
                nc.vector.reciprocal(out=r_, in_=r_)
                nmr = smp.tile([P, 1], F32, tag="nmr")
                nc.vector.tensor_scalar(out=nmr, in0=mv[:, 0:1], scalar1=r_,
                                        scalar2=-1.0, op0=OP.mult, op1=OP.mult)
                lnp = smp.tile([P, D], F32, tag="lnp", bufs=2)
                nc.vector.tensor_scalar(out=lnp, in0=y[:, tt, :], scalar1=r_,
                                        scalar2=nmr, op0=OP.mult, op1=OP.add)
                nc.vector.tensor_mul(out=lnp, in0=lnp, in1=flnw_bc)
                nc.vector.tensor_add(out=lnp, in0=lnp, in1=flnb_bc)
                nc.sync.dma_start(out=out_d[tt * P:(tt + 1) * P, :], in_=lnp)
    return nc


_NC_CACHE = {}


def _get_nc():
    if "nc" not in _NC_CACHE:
        nc = bacc.Bacc()
        _build(nc)
        nc.finalize()
        _NC_CACHE["nc"] = nc
    return _NC_CACHE["nc"]


def _prep_weights(inp):
    f = np.float32

    def a(x):
        return np.ascontiguousarray(np.asarray(x, dtype=f))

    out = {}
    # FFN1: fold ln gamma/beta into w1/b1
    w1 = a(inp["ffn1_w1"]); lw = a(inp["ffn1_ln_w"]); lb = a(inp["ffn1_ln_b"])
    out["w1f"] = a(w1 * lw[:, None])
    b1 = a(inp["ffn1_b1"]) + lb @ w1
    out["b1f"] = a(b1.reshape(DF_T, P).T)
    out["w2"] = a(inp["ffn1_w2"])
    out["b2h"] = a(0.5 * a(inp["ffn1_b2"]))
    # attention
    qkvw = a(inp["qkv_w"]); alw = a(inp["attn_ln_w"]); alb = a(inp["attn_ln_b"])
    qkvf = qkvw * alw[:, None]
    qkvb = a(inp["qkv_b"]) + alb @ qkvw
    scale = np.float32(DH ** -0.5)
    qkvf[:, :D] *= scale
    out["qkvw"] = a(qkvf)
    out["qb"] = a((qkvb[:D] * scale).reshape(4, P).T)
    out["kb"] = a(qkvb[D:2 * D].reshape(4, P).T)
    out["vb"] = a(qkvb[2 * D:].reshape(8, 64).T)
    out["outw"] = a(inp["out_w"])
    out["outb"] = a(inp["out_b"])
    # conv module
    pw1 = a(inp["pw1_w"]); clw = a(inp["conv_ln_w"]); clb = a(inp["conv_ln_b"])
    out["pw1t"] = a((pw1 * clw[None, :]).T)
    pb = a(inp["pw1_b"]) + pw1 @ clb
    out["ba1"] = a(pb[:DC].reshape(DC_T, P).T)
    out["ba2"] = a(pb[DC:].reshape(DC_T, P).T)
    dw = a(inp["dw_w"]).reshape(DC, KTAP)
    dg = np.zeros((DC_T, KTAP, P, P), dtype=f)
    idx = np.arange(P)
    for ct in range(DC_T):
        for j in range(KTAP):
            dg[ct, j, idx, idx] = dw[ct * P:(ct + 1) * P, j]
    out["dwdiag"] = a(dg)
    out["dwb"] = a(a(inp["dw_b"]).reshape(DC_T, P).T)
    out["gnw"] = a(a(inp["gn_w"]).reshape(DC_T, P).T)
    out["gnb"] = a(a(inp["gn_b"]).reshape(DC_T, P).T)
    out["pw2t"] = a(a(inp["pw2_w"]).T)
    out["pw2b"] = a(inp["pw2_b"])
    # FFN2
    w12 = a(inp["ffn2_w1"]); lw2 = a(inp["ffn2_ln_w"]); lb2 = a(inp["ffn2_ln_b"])
    out["w1f2"] = a(w12 * lw2[:, None])
    b12 = a(inp["ffn2_b1"]) + lb2 @ w12
    out["b1f2"] = a(b12.reshape(DF_T, P).T)
    out["w22"] = a(inp["ffn2_w2"])
    out["b2h2"] = a(0.5 * a(inp["ffn2_b2"]))
    out["flnw"] = a(inp["final_ln_w"])
    out["flnb"] = a(inp["final_ln_b"])
    out["ident"] = np.eye(P, dtype=f)
    out["zeros"] = np.zeros(128, dtype=f)
    # attention masks: [4, 128, 384]; additive
    masks = np.full((4, P, 384), NEG, dtype=f)
    i = np.arange(P)[:, None]
    p = np.arange(384)[None, :]
    w2_ = WIN // 2
    # qh0 interior: valid p in [i, i+64]
    masks[0][(p >= i) & (p <= i + 2 * w2_)] = 0.0
    # qh1 interior: valid p in [128+i, 192+i]
    masks[1][(p >= P + i) & (p <= P + i + 2 * w2_)] = 0.0
    # left edge (qp0,qh0): additionally p >= 32 (keys >= 0)
    masks[2][(p >= i) & (p <= i + 2 * w2_) & (p >= PAD)] = 0.0
    # right edge (qp3,qh1): additionally p < 288 (keys < 1024)
    masks[3][(p >= P + i) & (p <= P + i + 2 * w2_) & (p < 288)] = 0.0
    out["masks"] = a(masks)
    return out


def kernel(**inputs):
    x = np.asarray(inputs["x"], dtype=np.float32)
    assert x.shape == (B, T, D)
    weights = _prep_weights(inputs)
    nc = _get_nc()
    in_maps = []
    for i in range(N_CORES):
        m = dict(weights)
        m["x"] = np.ascontiguousarray(x[i])
        in_maps.append(m)
    res = run_bass_kernel_spmd(nc, in_maps, core_ids=list(range(N_CORES)))
    outs = [res.results[i]["y_out"] for i in range(N_CORES)]
    return np.stack(outs, axis=0).astype(np.float32)


if __name__ == "__main__":
    rng = np.random.default_rng(0)
    pass



# revision 24
# speedup vs baseline: 1.1095x; 1.1095x over previous
"""ConformerBlock Trainium2 kernel.

Data-parallel over batch: B=8 = one batch element per NeuronCore, no
collectives (every module in the block is per-sample, including the
GroupNorm which normalizes over (C,T) of each sample).

Per-core layout strategy:
  - residual `y` kept time-major [T=1024, D=512] as [128, 8, 512] SBUF tile
  - LayerNorm stats via bn_stats per 128-row time tile; the LN gamma/beta
    are folded into the *following* matmul's weights on the host, so on
    chip LN is just (x - m) * rsqrt(v+eps) (one scalar-engine op)
  - the normalized tile is PE-transposed to feature-major [D, T] which
    feeds every matmul (weights stationary [K,M], activations moving
    [K,N<=512]); all matmuls run in float32r (FP22 multiply, FP32
    accumulate, full PE rate at N>=256)
  - windowed attention (|i-j|<=32): per 256-query pair, a 384-wide key
    window; scores + additive mask + exp (no max-sub: scores are O(1)),
    row-normalize, PE-transpose the probs, then AV with time-major V
  - depthwise conv K=31 runs on the PE as 31 accumulating matmuls with
    per-tap diagonal weight matrices (prebuilt on host, streamed from
    DRAM) against sliding windows of the zero-padded GLU output
"""

import numpy as np
import ml_dtypes

import concourse.bass as bass
import concourse.bacc as bacc
import concourse.tile as tile
from concourse import mybir
from concourse.bass_utils import run_bass_kernel_spmd
from concourse import bass_isa

F32 = mybir.dt.float32
F32R = mybir.dt.float32r
BF16 = mybir.dt.bfloat16
NP_BF16 = ml_dtypes.bfloat16
AF = mybir.ActivationFunctionType
OP = mybir.AluOpType
AX = mybir.AxisListType

B, T, D, H, KTAP, WIN = 8, 1024, 512, 8, 31, 64
DF = 4 * D            # 2048 ffn hidden
DC = 2 * D            # 1024 conv channels
DH = D // H           # 64
EPS = 1e-5
P = 128
TT_N = T // P         # 8 time tiles
D_T = D // P          # 4
DF_T = DF // P        # 16
DC_T = DC // P        # 8
PAD = 32              # ln_t / kpad leading pad
KP_W = PAD + T + 96   # 1152 padded time width (feature-major)
CPAD = 15             # conv halo
G_W = T + 2 * CPAD    # 1054
NEG = -30000.0

N_CORES = 8


def _dram_vec_bcast_ap(dram_ap, n):
    """AP reading a [n] dram vector broadcast across 128 partitions."""
    return bass.AP(tensor=dram_ap.tensor, offset=dram_ap.offset,
                   ap=[[0, P], [1, n]])


def _build(nc):
    dp = nc.declare_dram_parameter
    x_d = dp("x", [T, D], F32, isOutput=False)
    w1f_d = dp("w1f", [D, DF], BF16, isOutput=False)
    b1f_d = dp("b1f", [P, DF_T], F32, isOutput=False)
    w2_d = dp("w2", [DF, D], BF16, isOutput=False)
    b2h_d = dp("b2h", [D], F32, isOutput=False)
    qkvw_d = dp("qkvw", [D, 3 * D], BF16, isOutput=False)
    qb_d = dp("qb", [P, 4], F32, isOutput=False)
    kb_d = dp("kb", [P, 4], F32, isOutput=False)
    vb_d = dp("vb", [64, 8], F32, isOutput=False)
    outw_d = dp("outw", [D, D], BF16, isOutput=False)
    outb_d = dp("outb", [D], F32, isOutput=False)
    pw1t_d = dp("pw1t", [D, 2 * DC], BF16, isOutput=False)
    ba1_d = dp("ba1", [P, DC_T], F32, isOutput=False)
    ba2_d = dp("ba2", [P, DC_T], F32, isOutput=False)
    dwdiag_d = dp("dwdiag", [DC_T, KTAP, P, P], BF16, isOutput=False)
    dwb_d = dp("dwb", [P, DC_T], F32, isOutput=False)
    gnw_d = dp("gnw", [P, DC_T], F32, isOutput=False)
    gnb_d = dp("gnb", [P, DC_T], F32, isOutput=False)
    pw2t_d = dp("pw2t", [DC, D], BF16, isOutput=False)
    pw2b_d = dp("pw2b", [D], F32, isOutput=False)
    w1f2_d = dp("w1f2", [D, DF], BF16, isOutput=False)
    b1f2_d = dp("b1f2", [P, DF_T], F32, isOutput=False)
    w22_d = dp("w22", [DF, D], BF16, isOutput=False)
    b2h2_d = dp("b2h2", [D], F32, isOutput=False)
    flnw_d = dp("flnw", [D], F32, isOutput=False)
    flnb_d = dp("flnb", [D], F32, isOutput=False)
    ident_d = dp("ident", [P, P], BF16, isOutput=False)
    zeros_d = dp("zeros", [128], BF16, isOutput=False)
    masks_d = dp("masks", [4, P, 384], F32, isOutput=False)
    out_d = dp("y_out", [T, D], F32, isOutput=True)

    with tile.TileContext(nc) as tc:
        with (
            tc.tile_pool(name="const", bufs=1) as cpool,
            tc.tile_pool(name="resid", bufs=1) as rpool,
            tc.tile_pool(name="lnt", bufs=1) as lpool,
            tc.tile_pool(name="big", bufs=1) as bigp,
            tc.tile_pool(name="w2res", bufs=1) as w2p,
            tc.tile_pool(name="wbig", bufs=1) as wbp,
            tc.tile_pool(name="med", bufs=1) as medp,
            tc.tile_pool(name="wd", bufs=2) as wdp,
            tc.tile_pool(name="small", bufs=4) as smp,
            tc.tile_pool(name="smx", bufs=4) as smxp,
            tc.tile_pool(name="attnT", bufs=2) as atp,
            tc.tile_pool(name="psA", bufs=2, space="PSUM") as psA,
            tc.tile_pool(name="psB", bufs=4, space="PSUM") as psB,
            tc.tile_pool(name="psC", bufs=2, space="PSUM") as psC,
        ):
            # ---------------- constants ----------------
            ident = cpool.tile([P, P], BF16, tag="ident")
            nc.sync.dma_start(out=ident, in_=ident_d[:, :])
            identR = ident
            eps_t = cpool.tile([P, 1], F32, tag="eps")
            nc.vector.memset(eps_t, EPS)

            def bcast_tile(dram_ap, tag):
                t_ = cpool.tile([P, D], F32, tag=tag)
                nc.sync.dma_start(out=t_, in_=_dram_vec_bcast_ap(dram_ap, D))
                return t_

            b2h_bc = bcast_tile(b2h_d[:], "b2h")
            outb_bc = bcast_tile(outb_d[:], "outb")
            pw2b_bc = bcast_tile(pw2b_d[:], "pw2b")
            b2h2_bc = bcast_tile(b2h2_d[:], "b2h2")
            flnw_bc = bcast_tile(flnw_d[:], "flnw")
            flnb_bc = bcast_tile(flnb_d[:], "flnb")

            def c2d(dram, n, tag):
                t_ = cpool.tile([P, n], F32, tag=tag)
                nc.sync.dma_start(out=t_, in_=dram[:, :])
                return t_

            b1f_t = c2d(b1f_d, DF_T, "b1f")
            qb_t = c2d(qb_d, 4, "qb")
            kb_t = c2d(kb_d, 4, "kb")
            vb_t = cpool.tile([64, 8], F32, tag="vb")
            nc.sync.dma_start(out=vb_t, in_=vb_d[:, :])
            ba1_t = c2d(ba1_d, DC_T, "ba1")
            ba2_t = c2d(ba2_d, DC_T, "ba2")
            dwb_t = c2d(dwb_d, DC_T, "dwb")
            gnw_t = c2d(gnw_d, DC_T, "gnw")
            gnb_t = c2d(gnb_d, DC_T, "gnb")
            b1f2_t = c2d(b1f2_d, DF_T, "b1f2")

            masks_t = cpool.tile([P, 4, 384], F32, tag="masks")
            for i in range(4):
                nc.sync.dma_start(out=masks_t[:, i, :], in_=masks_d[i, :, :])
            # mask index: 0=mid qh0, 1=mid qh1, 2=left (qp0,qh0), 3=right (qp3,qh1)

            zeros_ap = zeros_d[:]

            def zfill(out_ap, n1, n2):
                nc.sync.dma_start(out=out_ap, in_=bass.AP(
                    tensor=zeros_ap.tensor, offset=zeros_ap.offset,
                    ap=[[0, P], [0, n1], [1, n2]]))

            # ---------------- residual load ----------------
            y = rpool.tile([P, TT_N, D], F32, tag="y")
            nc.sync.dma_start(out=y,
                              in_=x_d.rearrange("(a p) d -> p a d", p=P))

            # ---------------- layernorm -> feature-major ----------------
            def layer_norm_t(need_pad):
                """LN of y (no gamma/beta: folded into next weights), transposed
                into a fresh feature-major [128, D_T, KP_W] tile (data at col
                PAD..PAD+T)."""
                ln_t = lpool.tile([P, D_T, KP_W], BF16, tag="lnt")
                if need_pad:
                    zfill(ln_t[:, :, 0:PAD], D_T, PAD)
                    zfill(ln_t[:, :, PAD + T:KP_W], D_T, KP_W - PAD - T)
                for tt in range(TT_N):
                    mv = smp.tile([P, 2], F32, tag="mv")
                    st6 = smp.tile([P, 6], F32, tag="st6")
                    nc.vector.bn_stats(out=st6, in_=y[:, tt, :])
                    nc.vector.bn_aggr(out=mv, in_=st6)
                    r_ = smp.tile([P, 1], F32, tag="r")
                    nc.scalar.activation(out=r_, in_=mv[:, 1:2], func=AF.Sqrt,
                                         bias=eps_t, scale=1.0)
                    nc.vector.reciprocal(out=r_, in_=r_)
                    nmr = smp.tile([P, 1], F32, tag="nmr")
                    nc.vector.tensor_scalar(out=nmr, in0=mv[:, 0:1], scalar1=r_,
                                            scalar2=-1.0, op0=OP.mult, op1=OP.mult)
                    lnp = smp.tile([P, D], BF16, tag="lnp", bufs=2)
                    nc.vector.tensor_scalar(out=lnp, in0=y[:, tt, :], scalar1=r_,
                                            scalar2=nmr, op0=OP.mult, op1=OP.add)
                    tp = psB.tile([P, D], BF16, tag="tp")
                    for dt in range(D_T):
                        nc.tensor.transpose(tp[:, dt * P:(dt + 1) * P],
                                            lnp[:, dt * P:(dt + 1) * P],
                                            identR)
                        nc.scalar.activation(
                            out=ln_t[:, dt, PAD + tt * P:PAD + (tt + 1) * P],
                            in_=tp[:, dt * P:(dt + 1) * P],
                            func=AF.Copy)
                return ln_t

            # ---------------- FFN (macaron half-residual) ----------------
            def ffn(w1_dram, b1_tile, w2_dram, b2h_bcast):
                ln_t = layer_norm_t(False)
                w2r = w2p.tile([P, DF_T, D], BF16, tag="w2res")
                for kt in range(DF_T):
                    nc.sync.dma_start(out=w2r[:, kt, :],
                                      in_=w2_dram[kt * P:(kt + 1) * P, :])
                w1t = wbp.tile([P, D_T, DF], BF16, tag="wbig")
                for kt in range(D_T):
                    nc.sync.dma_start(out=w1t[:, kt, :],
                                      in_=w1_dram[kt * P:(kt + 1) * P, :])
                for th in range(2):
                    h1 = bigp.tile([P, DF_T, D], BF16, tag="big")
                    for ft in range(DF_T):
                        ps = psA.tile([P, D], F32, tag="mm")
                        for kt in range(D_T):
                            nc.tensor.matmul(
                                ps,
                                lhsT=w1t[:, kt, ft * P:(ft + 1) * P],
                                rhs=ln_t[:, kt, PAD + th * D:PAD + (th + 1) * D],
                                start=(kt == 0), stop=(kt == D_T - 1))
                        nc.scalar.activation(out=h1[:, ft, :], in_=ps, func=AF.Silu,
                                             bias=b1_tile[:, ft:ft + 1], scale=1.0)
                    for tc in range(4):
                        ps2 = psA.tile([P, D], F32, tag="mm")
                        for kt in range(DF_T):
                            nc.tensor.matmul(
                                ps2,
                                lhsT=h1[:, kt, tc * P:(tc + 1) * P],
                                rhs=w2r[:, kt, :],
                                start=(kt == 0), stop=(kt == DF_T - 1))
                        g_tc = th * 4 + tc
                        nc.vector.scalar_tensor_tensor(
                            out=y[:, g_tc, :], in0=ps2, scalar=0.5,
                            in1=y[:, g_tc, :], op0=OP.mult, op1=OP.add)
                        nc.vector.tensor_add(out=y[:, g_tc, :], in0=y[:, g_tc, :],
                                             in1=b2h_bcast)

            # ================= FFN1 =================
            ffn(w1f_d, b1f_t, w2_d, b2h_bc)

            # ================= attention =================
            ln_t = layer_norm_t(True)
            qkvt = wbp.tile([P, D_T, 3 * D], BF16, tag="wbig")
            for kt in range(D_T):
                nc.sync.dma_start(out=qkvt[:, kt, :],
                                  in_=qkvw_d[kt * P:(kt + 1) * P, :])
            qk = bigp.tile([P, 4 * T + 4 * KP_W], BF16, tag="big")
            q_all = qk[:, 0:4 * T].rearrange("p (h t) -> p h t", h=4)
            kpad = qk[:, 4 * T:4 * T + 4 * KP_W].rearrange("p (h t) -> p h t", h=4)
            zfill(kpad[:, :, 0:PAD], 4, PAD)
            zfill(kpad[:, :, PAD + T:KP_W], 4, KP_W - PAD - T)
            # q, k: feature-major [head-pair 128, T]
            for hp in range(4):
                for tn in range(2):
                    psq = psA.tile([P, D], F32, tag="mm")
                    for kt in range(D_T):
                        nc.tensor.matmul(
                            psq,
                            lhsT=qkvt[:, kt, hp * P:(hp + 1) * P],
                            rhs=ln_t[:, kt, PAD + tn * D:PAD + (tn + 1) * D],
                            start=(kt == 0), stop=(kt == D_T - 1))
                    nc.scalar.activation(out=q_all[:, hp, tn * D:(tn + 1) * D],
                                         in_=psq, func=AF.Identity,
                                         bias=qb_t[:, hp:hp + 1], scale=1.0)
                    psk = psA.tile([P, D], F32, tag="mm")
                    for kt in range(D_T):
                        nc.tensor.matmul(
                            psk,
                            lhsT=qkvt[:, kt, D + hp * P:D + (hp + 1) * P],
                            rhs=ln_t[:, kt, PAD + tn * D:PAD + (tn + 1) * D],
                            start=(kt == 0), stop=(kt == D_T - 1))
                    nc.scalar.activation(
                        out=kpad[:, hp, PAD + tn * D:PAD + (tn + 1) * D],
                        in_=psk, func=AF.Identity,
                        bias=kb_t[:, hp:hp + 1], scale=1.0)
            # v: time-major, stored at +32 row offset (9 slots of 128)
            vpad = w2p.tile([P, 9, D], BF16, tag="w2res")
            for vt in range(9):
                psv = psA.tile([P, D], F32, tag="mm")
                for kt in range(D_T):
                    nc.tensor.matmul(
                        psv,
                        lhsT=ln_t[:, kt, vt * P:(vt + 1) * P],
                        rhs=qkvt[:, kt, 2 * D:3 * D],
                        start=(kt == 0), stop=(kt == D_T - 1))
                nc.scalar.activation(out=vpad[:, vt, :], in_=psv, func=AF.Copy)

            o_t = [medp.tile([64, T], BF16, tag=f"med{i}", name=f"o_t{i}")
                   for i in range(8)]
            for hp in range(4):
                for qp in range(4):
                    for hi in range(2):
                        h = 2 * hp + hi
                        base = hi * 64
                        av = psB.tile([64, 256], F32, tag="tp", name=f"av{h}_{qp}")
                        smx_h = []
                        for qh in range(2):
                            sc = psC.tile([P, 384], F32, tag="sc")
                            nc.tensor.matmul(
                                sc,
                                lhsT=q_all[base:base + 64, hp,
                                           qp * 256 + qh * P:qp * 256 + (qh + 1) * P],
                                rhs=kpad[base:base + 64, hp,
                                         qp * 256:qp * 256 + 384],
                                start=True, stop=True)
                            if qh == 0:
                                mi = 2 if qp == 0 else 0
                            else:
                                mi = 3 if qp == 3 else 1
                            smx = smxp.tile([P, 384], F32, tag="smx")
                            nc.vector.tensor_add(out=smx, in0=sc,
                                                 in1=masks_t[:, mi, :])
                            lsum = smp.tile([P, 1], F32, tag="lsum")
                            nc.scalar.activation(out=smx, in_=smx, func=AF.Exp,
                                                 accum_out=lsum)
                            rr = smp.tile([P, 1], F32, tag="rr")
                            nc.vector.reciprocal(out=rr, in_=lsum)
                            smxb = smxp.tile([P, 384], BF16, tag="smxb")
                            nc.vector.tensor_scalar(out=smxb, in0=smx, scalar1=rr,
                                                    scalar2=None, op0=OP.mult)
                            smx_h.append(smxb)
                        for sb in range(3):
                            tp2 = psB.tile([P, 256], BF16, tag="tp")
                            for qh in range(2):
                                nc.tensor.transpose(
                                    tp2[:, qh * P:(qh + 1) * P],
                                    smx_h[qh][:, sb * P:(sb + 1) * P],
                                    identR)
                            at = atp.tile([P, 256], BF16, tag="at")
                            nc.scalar.activation(out=at, in_=tp2, func=AF.Copy)
                            vt = qp * 2 + sb
                            nc.tensor.matmul(
                                av,
                                lhsT=vpad[:, vt, h * DH:(h + 1) * DH],
                                rhs=at,
                                start=(sb == 0), stop=(sb == 2))
                        nc.scalar.activation(
                            out=o_t[h][:, qp * 256:(qp + 1) * 256],
                            in_=av, func=AF.Identity,
                            bias=vb_t[:, h:h + 1], scale=1.0)
            # out projection + residual
            outwt = wbp.tile([64, 8, D], BF16, tag="wbig")
            nc.sync.dma_start(out=outwt,
                              in_=outw_d.rearrange("(a p) d -> p a d", p=64))
            for tc in range(TT_N):
                pso = psA.tile([P, D], F32, tag="mm")
                for h in range(8):
                    nc.tensor.matmul(
                        pso,
                        lhsT=o_t[h][:, tc * P:(tc + 1) * P],
                        rhs=outwt[:, h, :],
                        start=(h == 0), stop=(h == 7))
                nc.vector.tensor_add(out=y[:, tc, :], in0=y[:, tc, :], in1=pso)
                nc.vector.tensor_add(out=y[:, tc, :], in0=y[:, tc, :], in1=outb_bc)

            # ================= conv module =================
            ln_t = layer_norm_t(False)
            pw1tt = wbp.tile([P, D_T, 2 * DC], BF16, tag="wbig")
            for kt in range(D_T):
                nc.sync.dma_start(out=pw1tt[:, kt, :],
                                  in_=pw1t_d[kt * P:(kt + 1) * P, :])
            g = bigp.tile([P, DC_T, G_W], BF16, tag="big")
            zfill(g[:, :, 0:CPAD], DC_T, CPAD)
            zfill(g[:, :, CPAD + T:G_W], DC_T, CPAD)
            for ct in range(DC_T):
                for tn in range(2):
                    ps_a2 = psA.tile([P, D], F32, tag="mm")
                    for kt in range(D_T):
                        nc.tensor.matmul(
                            ps_a2,
                            lhsT=pw1tt[:, kt, DC + ct * P:DC + (ct + 1) * P],
                            rhs=ln_t[:, kt, PAD + tn * D:PAD + (tn + 1) * D],
                            start=(kt == 0), stop=(kt == D_T - 1))
                    sig = smp.tile([P, D], F32, tag="sig", bufs=2)
                    nc.scalar.activation(out=sig, in_=ps_a2, func=AF.Sigmoid,
                                         bias=ba2_t[:, ct:ct + 1], scale=1.0)
                    ps_a1 = psA.tile([P, D], F32, tag="mm")
                    for kt in range(D_T):
                        nc.tensor.matmul(
                            ps_a1,
                            lhsT=pw1tt[:, kt, ct * P:(ct + 1) * P],
                            rhs=ln_t[:, kt, PAD + tn * D:PAD + (tn + 1) * D],
                            start=(kt == 0), stop=(kt == D_T - 1))
                    nc.vector.scalar_tensor_tensor(
                        out=g[:, ct, CPAD + tn * D:CPAD + (tn + 1) * D],
                        in0=ps_a1, scalar=ba1_t[:, ct:ct + 1], in1=sig,
                        op0=OP.add, op1=OP.mult)
            # depthwise conv: 31 accumulating diag matmuls, sliding rhs window
            cv = [medp.tile([P, T], BF16, tag=f"med{i}", name=f"cv{i}") for i in range(DC_T)]
            for ct in range(DC_T):
                pcs = [psA.tile([P, D], F32, tag="mm", name=f"pcs{ct}_{k}") for k in range(2)]
                for j in range(KTAP):
                    dg = wdp.tile([P, P], BF16, tag="wd")
                    nc.sync.dma_start(out=dg, in_=dwdiag_d[ct, j, :, :])
                    for tn in range(2):
                        nc.tensor.matmul(
                            pcs[tn],
                            lhsT=dg,
                            rhs=g[:, ct, j + tn * D:j + (tn + 1) * D],
                            start=(j == 0), stop=(j == KTAP - 1))
                for tn in range(2):
                    nc.scalar.activation(out=cv[ct][:, tn * D:(tn + 1) * D],
                                         in_=pcs[tn], func=AF.Identity,
                                         bias=dwb_t[:, ct:ct + 1], scale=1.0)
            # GroupNorm(1 group over C,T) stats
            stats_pk = smp.tile([P, 16], F32, tag="stpk")
            for ct in range(DC_T):
                st = smp.tile([P, 2, 6], F32, tag="st26")
                nc.vector.bn_stats(out=st[:, 0, :], in_=cv[ct][:, 0:D])
                nc.vector.bn_stats(out=st[:, 1, :], in_=cv[ct][:, D:T])
                mv = smp.tile([P, 2], F32, tag="mv")
                nc.vector.bn_aggr(out=mv, in_=st)
                nc.vector.tensor_copy(out=stats_pk[:, ct:ct + 1], in_=mv[:, 0:1])
                nc.vector.scalar_tensor_tensor(
                    out=stats_pk[:, 8 + ct:9 + ct], in0=mv[:, 0:1],
                    scalar=mv[:, 0:1], in1=mv[:, 1:2], op0=OP.mult, op1=OP.add)
            red = smp.tile([P, 16], F32, tag="gred")
            nc.gpsimd.partition_all_reduce(red, stats_pk, channels=P,
                                           reduce_op=bass_isa.ReduceOp.add)
            sums = smp.tile([P, 2], F32, tag="sums")
            nc.vector.tensor_reduce(out=sums,
                                    in_=red.rearrange("p (a b) -> p a b", a=2),
                                    axis=AX.X, op=OP.add)
            mq = smp.tile([P, 2], F32, tag="mq")  # [mu, E[x^2]] on every partition
            nc.vector.tensor_scalar(out=mq, in0=sums, scalar1=1.0 / DC,
                                    scalar2=None, op0=OP.mult)
            # var = E[x^2] - mu^2
            var_t = smp.tile([P, 1], F32, tag="var")
            nc.vector.tensor_scalar(out=var_t, in0=mq[:, 0:1], scalar1=mq[:, 0:1],
                                    scalar2=-1.0, op0=OP.mult, op1=OP.mult)
            nc.vector.tensor_add(out=var_t, in0=var_t, in1=mq[:, 1:2])
            rstd = smp.tile([P, 1], F32, tag="rstd")
            nc.scalar.activation(out=rstd, in_=var_t, func=AF.Sqrt,
                                 bias=eps_t, scale=1.0)
            nc.vector.reciprocal(out=rstd, in_=rstd)
            # per-channel-tile scale/shift + SiLU, then pw2 + residual
            pw2tt = w2p.tile([P, DC_T, D], BF16, tag="w2res")
            for kt in range(DC_T):
                nc.sync.dma_start(out=pw2tt[:, kt, :],
                                  in_=pw2t_d[kt * P:(kt + 1) * P, :])
            for ct in range(DC_T):
                s_c = smp.tile([P, 1], F32, tag="s_c")
                nc.vector.tensor_scalar(out=s_c, in0=gnw_t[:, ct:ct + 1],
                                        scalar1=rstd, scalar2=None,
                                        op0=OP.mult)
                t_c = smp.tile([P, 1], F32, tag="t_c")
                nc.vector.tensor_scalar(out=t_c, in0=s_c, scalar1=mq[:, 0:1],
                                        scalar2=-1.0, op0=OP.mult, op1=OP.mult)
                nc.vector.tensor_add(out=t_c, in0=t_c, in1=gnb_t[:, ct:ct + 1])
                nc.scalar.activation(out=cv[ct], in_=cv[ct], func=AF.Silu,
                                     bias=t_c, scale=s_c)
            for tc in range(TT_N):
                psp = psA.tile([P, D], F32, tag="mm")
                for kt in range(DC_T):
                    nc.tensor.matmul(
                        psp,
                        lhsT=cv[kt][:, tc * P:(tc + 1) * P],
                        rhs=pw2tt[:, kt, :],
                        start=(kt == 0), stop=(kt == DC_T - 1))
                nc.vector.tensor_add(out=y[:, tc, :], in0=y[:, tc, :], in1=psp)
                nc.vector.tensor_add(out=y[:, tc, :], in0=y[:, tc, :], in1=pw2b_bc)

            # ================= FFN2 =================
            ffn(w1f2_d, b1f2_t, w22_d, b2h2_bc)

            # ================= final LN + store =================
            for tt in range(TT_N):
                mv = smp.tile([P, 2], F32, tag="mv")
                st6 = smp.tile([P, 6], F32, tag="st6")
                nc.vector.bn_stats(out=st6, in_=y[:, tt, :])
                nc.vector.bn_aggr(out=mv, in_=st6)
                r_ = smp.tile([P, 1], F32, tag="r")
                nc.scalar.activation(out=r_, in_=mv[:, 1:2], func=AF.Sqrt,
                                     bias=eps_t, scale=1.0)
                nc.vector.reciprocal(out=r_, in_=r_)
                nmr = smp.tile([P, 1], F32, tag="nmr")
                nc.vector.tensor_scalar(out=nmr, in0=mv[:, 0:1], scalar1=r_,
                                        scalar2=-1.0, op0=OP.mult, op1=OP.mult)
                lnp = smp.tile([P, D], F32, tag="lnp", bufs=2)
                nc.vector.tensor_scalar(out=lnp, in0=y[:, tt, :], scalar1=r_,
                                        scalar2=nmr, op0=OP.mult, op1=OP.add)
                nc.vector.tensor_mul(out=lnp, in0=lnp, in1=flnw_bc)
                nc.vector.tensor_add(out=lnp, in0=lnp, in1=flnb_bc)
                nc.sync.dma_start(out=out_d[tt * P:(tt + 1) * P, :], in_=lnp)
    return nc


_NC_CACHE = {}


def _get_nc():
    if "nc" not in _NC_CACHE:
        nc = bacc.Bacc()
        _build(nc)
        nc.finalize()
        _NC_CACHE["nc"] = nc
    return _NC_CACHE["nc"]


def _prep_weights(inp):
    f = np.float32

    def a(x):
        return np.ascontiguousarray(np.asarray(x, dtype=f))

    def b(x):
        return np.ascontiguousarray(np.asarray(x, dtype=f).astype(NP_BF16))

    out = {}
    # FFN1: fold ln gamma/beta into w1/b1
    w1 = a(inp["ffn1_w1"]); lw = a(inp["ffn1_ln_w"]); lb = a(inp["ffn1_ln_b"])
    out["w1f"] = b(w1 * lw[:, None])
    b1 = a(inp["ffn1_b1"]) + lb @ w1
    out["b1f"] = a(b1.reshape(DF_T, P).T)
    out["w2"] = b(inp["ffn1_w2"])
    out["b2h"] = a(0.5 * a(inp["ffn1_b2"]))
    # attention
    qkvw = a(inp["qkv_w"]); alw = a(inp["attn_ln_w"]); alb = a(inp["attn_ln_b"])
    qkvf = qkvw * alw[:, None]
    qkvb = a(inp["qkv_b"]) + alb @ qkvw
    scale = np.float32(DH ** -0.5)
    qkvf[:, :D] *= scale
    out["qkvw"] = b(qkvf)
    out["qb"] = a((qkvb[:D] * scale).reshape(4, P).T)
    out["kb"] = a(qkvb[D:2 * D].reshape(4, P).T)
    out["vb"] = a(qkvb[2 * D:].reshape(8, 64).T)
    out["outw"] = b(inp["out_w"])
    out["outb"] = a(inp["out_b"])
    # conv module
    pw1 = a(inp["pw1_w"]); clw = a(inp["conv_ln_w"]); clb = a(inp["conv_ln_b"])
    out["pw1t"] = b((pw1 * clw[None, :]).T)
    pb = a(inp["pw1_b"]) + pw1 @ clb
    out["ba1"] = a(pb[:DC].reshape(DC_T, P).T)
    out["ba2"] = a(pb[DC:].reshape(DC_T, P).T)
    dw = a(inp["dw_w"]).reshape(DC, KTAP)
    dg = np.zeros((DC_T, KTAP, P, P), dtype=f)
    idx = np.arange(P)
    for ct in range(DC_T):
        for j in range(KTAP):
            dg[ct, j, idx, idx] = dw[ct * P:(ct + 1) * P, j]
    out["dwdiag"] = b(dg)
    out["dwb"] = a(a(inp["dw_b"]).reshape(DC_T, P).T)
    out["gnw"] = a(a(inp["gn_w"]).reshape(DC_T, P).T)
    out["gnb"] = a(a(inp["gn_b"]).reshape(DC_T, P).T)
    out["pw2t"] = b(a(inp["pw2_w"]).T)
    out["pw2b"] = a(inp["pw2_b"])
    # FFN2
    w12 = a(inp["ffn2_w1"]); lw2 = a(inp["ffn2_ln_w"]); lb2 = a(inp["ffn2_ln_b"])
    out["w1f2"] = b(w12 * lw2[:, None])
    b12 = a(inp["ffn2_b1"]) + lb2 @ w12
    out["b1f2"] = a(b12.reshape(DF_T, P).T)
    out["w22"] = b(inp["ffn2_w2"])
    out["b2h2"] = a(0.5 * a(inp["ffn2_b2"]))
    out["flnw"] = a(inp["final_ln_w"])
    out["flnb"] = a(inp["final_ln_b"])
    out["ident"] = np.eye(P, dtype=f).astype(NP_BF16)
    out["zeros"] = np.zeros(128, dtype=NP_BF16)
    # attention masks: [4, 128, 384]; additive
    masks = np.full((4, P, 384), NEG, dtype=f)
    i = np.arange(P)[:, None]
    p = np.arange(384)[None, :]
    w2_ = WIN // 2
    # qh0 interior: valid p in [i, i+64]
    masks[0][(p >= i) & (p <= i + 2 * w2_)] = 0.0
    # qh1 interior: valid p in [128+i, 192+i]
    masks[1][(p >= P + i) & (p <= P + i + 2 * w2_)] = 0.0
    # left edge (qp0,qh0): additionally p >= 32 (keys >= 0)
    masks[2][(p >= i) & (p <= i + 2 * w2_) & (p >= PAD)] = 0.0
    # right edge (qp3,qh1): additionally p < 288 (keys < 1024)
    masks[3][(p >= P + i) & (p <= P + i + 2 * w2_) & (p < 288)] = 0.0
    out["masks"] = a(masks)
    return out


def kernel(**inputs):
    x = np.asarray(inputs["x"], dtype=np.float32)
    assert x.shape == (B, T, D)
    weights = _prep_weights(inputs)
    nc = _get_nc()
    in_maps = []
    for i in range(N_CORES):
        m = dict(weights)
        m["x"] = np.ascontiguousarray(x[i])
        in_maps.append(m)
    res = run_bass_kernel_spmd(nc, in_maps, core_ids=list(range(N_CORES)))
    outs = [res.results[i]["y_out"] for i in range(N_CORES)]
    return np.stack(outs, axis=0).astype(np.float32)


if __name__ == "__main__":
    rng = np.random.default_rng(0)
    pass



# revision 41
# speedup vs baseline: 1.2388x; 1.1165x over previous
"""ConformerBlock Trainium2 kernel.

Data-parallel over batch: B=8 = one batch element per NeuronCore, no
collectives (every module in the block is per-sample, including the
GroupNorm which normalizes over (C,T) of each sample).

Per-core layout strategy:
  - residual `y` kept time-major [T=1024, D=512] as [128, 8, 512] SBUF tile
  - LayerNorm stats via bn_stats per 128-row time tile; the LN gamma/beta
    are folded into the *following* matmul's weights on the host, so on
    chip LN is just (x - m) * rsqrt(v+eps) (one scalar-engine op)
  - the normalized tile is PE-transposed to feature-major [D, T] which
    feeds every matmul (weights stationary [K,M], activations moving
    [K,N<=512]); all matmuls run in float32r (FP22 multiply, FP32
    accumulate, full PE rate at N>=256)
  - windowed attention (|i-j|<=32): per 256-query pair, a 384-wide key
    window; scores + additive mask + exp (no max-sub: scores are O(1)),
    row-normalize, PE-transpose the probs, then AV with time-major V
  - depthwise conv K=31 runs on the PE as 31 accumulating matmuls with
    per-tap diagonal weight matrices (prebuilt on host, streamed from
    DRAM) against sliding windows of the zero-padded GLU output
"""

import numpy as np
import ml_dtypes

import concourse.bass as bass
import concourse.bacc as bacc
import concourse.tile as tile
from concourse import mybir
from concourse.bass_utils import run_bass_kernel_spmd
from concourse import bass_isa

F32 = mybir.dt.float32
F32R = mybir.dt.float32r
BF16 = mybir.dt.bfloat16
NP_BF16 = ml_dtypes.bfloat16
AF = mybir.ActivationFunctionType
OP = mybir.AluOpType
AX = mybir.AxisListType

B, T, D, H, KTAP, WIN = 8, 1024, 512, 8, 31, 64
DF = 4 * D            # 2048 ffn hidden
DC = 2 * D            # 1024 conv channels
DH = D // H           # 64
EPS = 1e-5
P = 128
TT_N = T // P         # 8 time tiles
D_T = D // P          # 4
DF_T = DF // P        # 16
DC_T = DC // P        # 8
PAD = 32              # ln_t / kpad leading pad
KP_W = PAD + T + 96   # 1152 padded time width (feature-major)
CPAD = 15             # conv halo
G_W = T + 2 * CPAD    # 1054
NEG = -30000.0

N_CORES = 8


def _dram_vec_bcast_ap(dram_ap, n):
    """AP reading a [n] dram vector broadcast across 128 partitions."""
    return bass.AP(tensor=dram_ap.tensor, offset=dram_ap.offset,
                   ap=[[0, P], [1, n]])


def _build(nc):
    dp = nc.declare_dram_parameter
    x_d = dp("x", [T, D], F32, isOutput=False)
    w1f_d = dp("w1f", [D, DF], BF16, isOutput=False)
    b1f_d = dp("b1f", [P, DF_T], F32, isOutput=False)
    w2_d = dp("w2", [DF, D], BF16, isOutput=False)
    b2h_d = dp("b2h", [D], F32, isOutput=False)
    qkvw_d = dp("qkvw", [D, 3 * D], BF16, isOutput=False)
    qb_d = dp("qb", [P, 4], F32, isOutput=False)
    kb_d = dp("kb", [P, 4], F32, isOutput=False)
    vb_d = dp("vb", [64, 8], F32, isOutput=False)
    outw_d = dp("outw", [D, D], BF16, isOutput=False)
    outb_d = dp("outb", [D], F32, isOutput=False)
    pw1t_d = dp("pw1t", [D, 2 * DC], BF16, isOutput=False)
    ba1_d = dp("ba1", [P, DC_T], F32, isOutput=False)
    ba2_d = dp("ba2", [P, DC_T], F32, isOutput=False)
    dww_d = dp("dww", [P, DC_T, KTAP], F32, isOutput=False)
    dwb_d = dp("dwb", [P, DC_T], F32, isOutput=False)
    gnw_d = dp("gnw", [P, DC_T], F32, isOutput=False)
    gnb_d = dp("gnb", [P, DC_T], F32, isOutput=False)
    pw2t_d = dp("pw2t", [DC, D], BF16, isOutput=False)
    pw2b_d = dp("pw2b", [D], F32, isOutput=False)
    w1f2_d = dp("w1f2", [D, DF], BF16, isOutput=False)
    b1f2_d = dp("b1f2", [P, DF_T], F32, isOutput=False)
    w22_d = dp("w22", [DF, D], BF16, isOutput=False)
    b2h2_d = dp("b2h2", [D], F32, isOutput=False)
    flnw_d = dp("flnw", [D], F32, isOutput=False)
    flnb_d = dp("flnb", [D], F32, isOutput=False)
    ident_d = dp("ident", [P, P], BF16, isOutput=False)
    zeros_d = dp("zeros", [128], BF16, isOutput=False)
    masks_d = dp("masks", [P, 4, 384], F32, isOutput=False)
    out_d = dp("y_out", [T, D], F32, isOutput=True)

    with tile.TileContext(nc) as tc:
        with (
            tc.tile_pool(name="const", bufs=1) as cpool,
            tc.tile_pool(name="resid", bufs=1) as rpool,
            tc.tile_pool(name="lnt", bufs=1) as lpool,
            tc.tile_pool(name="big", bufs=1) as bigp,
            tc.tile_pool(name="w2res", bufs=1) as w2p,
            tc.tile_pool(name="wbig", bufs=1) as wbp,
            tc.tile_pool(name="med", bufs=1) as medp,
            tc.tile_pool(name="wd", bufs=2) as wdp,
            tc.tile_pool(name="small", bufs=4) as smp,
            tc.tile_pool(name="smx", bufs=4) as smxp,
            tc.tile_pool(name="attnT", bufs=2) as atp,
            tc.tile_pool(name="psA", bufs=2, space="PSUM") as psA,
            tc.tile_pool(name="psB", bufs=4, space="PSUM") as psB,
            tc.tile_pool(name="psC", bufs=2, space="PSUM") as psC,
        ):
            # ---------------- constants ----------------
            ident = cpool.tile([P, P], BF16, tag="ident")
            nc.sync.dma_start(out=ident, in_=ident_d[:, :])
            identR = ident
            eps_t = cpool.tile([P, 1], F32, tag="eps")
            nc.vector.memset(eps_t, EPS)

            def bcast_tile(dram_ap, tag):
                t_ = cpool.tile([P, D], F32, tag=tag)
                nc.sync.dma_start(out=t_, in_=_dram_vec_bcast_ap(dram_ap, D))
                return t_

            b2h_bc = bcast_tile(b2h_d[:], "b2h")
            outb_bc = bcast_tile(outb_d[:], "outb")
            pw2b_bc = bcast_tile(pw2b_d[:], "pw2b")
            b2h2_bc = bcast_tile(b2h2_d[:], "b2h2")
            flnw_bc = bcast_tile(flnw_d[:], "flnw")
            flnb_bc = bcast_tile(flnb_d[:], "flnb")

            def c2d(dram, n, tag):
                t_ = cpool.tile([P, n], F32, tag=tag)
                nc.sync.dma_start(out=t_, in_=dram[:, :])
                return t_

            b1f_t = c2d(b1f_d, DF_T, "b1f")
            qb_t = c2d(qb_d, 4, "qb")
            kb_t = c2d(kb_d, 4, "kb")
            vb_t = cpool.tile([64, 8], F32, tag="vb")
            nc.sync.dma_start(out=vb_t, in_=vb_d[:, :])
            ba1_t = c2d(ba1_d, DC_T, "ba1")
            ba2_t = c2d(ba2_d, DC_T, "ba2")
            dwb_t = c2d(dwb_d, DC_T, "dwb")
            gnw_t = c2d(gnw_d, DC_T, "gnw")
            gnb_t = c2d(gnb_d, DC_T, "gnb")
            b1f2_t = c2d(b1f2_d, DF_T, "b1f2")
            dww_t = cpool.tile([P, DC_T, KTAP], F32, tag="dww")
            nc.sync.dma_start(out=dww_t, in_=dww_d[:, :, :])

            masks_t = cpool.tile([P, 4, 384], F32, tag="masks")
            nc.sync.dma_start(out=masks_t, in_=masks_d[:, :, :])
            # mask index: 0=mid qh0, 1=mid qh1, 2=left (qp0,qh0), 3=right (qp3,qh1)

            zeros_ap = zeros_d[:]

            def zfill(out_ap, n1, n2):
                nc.sync.dma_start(out=out_ap, in_=bass.AP(
                    tensor=zeros_ap.tensor, offset=zeros_ap.offset,
                    ap=[[0, P], [0, n1], [1, n2]]))

            # ---------------- residual load ----------------
            y = rpool.tile([P, TT_N, D], F32, tag="y")
            nc.sync.dma_start(out=y,
                              in_=x_d.rearrange("(a p) d -> p a d", p=P))

            # ---------------- layernorm -> feature-major ----------------
            def layer_norm_t(need_pad):
                """LN of y (no gamma/beta: folded into next weights), transposed
                into a fresh feature-major [128, D_T, KP_W] tile (data at col
                PAD..PAD+T)."""
                ln_t = lpool.tile([P, D_T, KP_W], BF16, tag="lnt")
                if need_pad:
                    zfill(ln_t[:, :, 0:PAD], D_T, PAD)
                    zfill(ln_t[:, :, PAD + T:KP_W], D_T, KP_W - PAD - T)
                for tt in range(TT_N):
                    mv = smp.tile([P, 2], F32, tag="mv")
                    st6 = smp.tile([P, 6], F32, tag="st6")
                    nc.vector.bn_stats(out=st6, in_=y[:, tt, :])
                    nc.vector.bn_aggr(out=mv, in_=st6)
                    r_ = smp.tile([P, 1], F32, tag="r")
                    nc.scalar.activation(out=r_, in_=mv[:, 1:2], func=AF.Sqrt,
                                         bias=eps_t, scale=1.0)
                    nc.vector.reciprocal(out=r_, in_=r_)
                    nmr = smp.tile([P, 1], F32, tag="nmr")
                    nc.vector.tensor_scalar(out=nmr, in0=mv[:, 0:1], scalar1=r_,
                                            scalar2=-1.0, op0=OP.mult, op1=OP.mult)
                    lnp = smp.tile([P, D], BF16, tag="lnp", bufs=2)
                    nc.vector.tensor_scalar(out=lnp, in0=y[:, tt, :], scalar1=r_,
                                            scalar2=nmr, op0=OP.mult, op1=OP.add)
                    tp = psB.tile([P, D], BF16, tag="tp")
                    for dt in range(D_T):
                        nc.tensor.transpose(tp[:, dt * P:(dt + 1) * P],
                                            lnp[:, dt * P:(dt + 1) * P],
                                            identR)
                        nc.scalar.activation(
                            out=ln_t[:, dt, PAD + tt * P:PAD + (tt + 1) * P],
                            in_=tp[:, dt * P:(dt + 1) * P],
                            func=AF.Copy)
                return ln_t

            # ---------------- FFN (macaron half-residual) ----------------
            def ffn(w1_dram, b1_tile, w2_dram, b2h_bcast):
                ln_t = layer_norm_t(False)
                w2r = w2p.tile([P, DF_T, D], BF16, tag="w2res")
                nc.sync.dma_start(out=w2r,
                                  in_=w2_dram.rearrange("(a p) d -> p a d", p=P))
                w1t = wbp.tile([P, D_T, DF], BF16, tag="wbig")
                nc.sync.dma_start(out=w1t,
                                  in_=w1_dram.rearrange("(a p) d -> p a d", p=P))
                for th in range(2):
                    h1 = bigp.tile([P, DF_T, D], BF16, tag="big")
                    for ft in range(DF_T):
                        ps = psA.tile([P, D], F32, tag="mm")
                        for kt in range(D_T):
                            nc.tensor.matmul(
                                ps,
                                lhsT=w1t[:, kt, ft * P:(ft + 1) * P],
                                rhs=ln_t[:, kt, PAD + th * D:PAD + (th + 1) * D],
                                start=(kt == 0), stop=(kt == D_T - 1))
                        nc.scalar.activation(out=h1[:, ft, :], in_=ps, func=AF.Silu,
                                             bias=b1_tile[:, ft:ft + 1], scale=1.0)
                    for tc in range(4):
                        ps2 = psA.tile([P, D], F32, tag="mm")
                        for kt in range(DF_T):
                            nc.tensor.matmul(
                                ps2,
                                lhsT=h1[:, kt, tc * P:(tc + 1) * P],
                                rhs=w2r[:, kt, :],
                                start=(kt == 0), stop=(kt == DF_T - 1))
                        g_tc = th * 4 + tc
                        nc.vector.scalar_tensor_tensor(
                            out=y[:, g_tc, :], in0=ps2, scalar=0.5,
                            in1=y[:, g_tc, :], op0=OP.mult, op1=OP.add)
                        nc.gpsimd.tensor_add(out=y[:, g_tc, :], in0=y[:, g_tc, :],
                                             in1=b2h_bcast)

            # ================= FFN1 =================
            ffn(w1f_d, b1f_t, w2_d, b2h_bc)

            # ================= attention =================
            ln_t = layer_norm_t(True)
            qkvt = wbp.tile([P, D_T, 3 * D], BF16, tag="wbig")
            nc.sync.dma_start(out=qkvt,
                              in_=qkvw_d.rearrange("(a p) d -> p a d", p=P))
            qk = bigp.tile([P, 4 * T + 4 * KP_W], BF16, tag="big")
            q_all = qk[:, 0:4 * T].rearrange("p (h t) -> p h t", h=4)
            kpad = qk[:, 4 * T:4 * T + 4 * KP_W].rearrange("p (h t) -> p h t", h=4)
            zfill(kpad[:, :, 0:PAD], 4, PAD)
            zfill(kpad[:, :, PAD + T:KP_W], 4, KP_W - PAD - T)
            # q, k: feature-major [head-pair 128, T]
            for hp in range(4):
                for tn in range(2):
                    psq = psA.tile([P, D], F32, tag="mm")
                    for kt in range(D_T):
                        nc.tensor.matmul(
                            psq,
                            lhsT=qkvt[:, kt, hp * P:(hp + 1) * P],
                            rhs=ln_t[:, kt, PAD + tn * D:PAD + (tn + 1) * D],
                            start=(kt == 0), stop=(kt == D_T - 1))
                    nc.scalar.activation(out=q_all[:, hp, tn * D:(tn + 1) * D],
                                         in_=psq, func=AF.Identity,
                                         bias=qb_t[:, hp:hp + 1], scale=1.0)
                    psk = psA.tile([P, D], F32, tag="mm")
                    for kt in range(D_T):
                        nc.tensor.matmul(
                            psk,
                            lhsT=qkvt[:, kt, D + hp * P:D + (hp + 1) * P],
                            rhs=ln_t[:, kt, PAD + tn * D:PAD + (tn + 1) * D],
                            start=(kt == 0), stop=(kt == D_T - 1))
                    nc.scalar.activation(
                        out=kpad[:, hp, PAD + tn * D:PAD + (tn + 1) * D],
                        in_=psk, func=AF.Identity,
                        bias=kb_t[:, hp:hp + 1], scale=1.0)
            # v: time-major, stored at +32 row offset (9 slots of 128)
            vpad = w2p.tile([P, 9, D], BF16, tag="w2res")
            for vt in range(9):
                psv = psA.tile([P, D], F32, tag="mm")
                for kt in range(D_T):
                    nc.tensor.matmul(
                        psv,
                        lhsT=ln_t[:, kt, vt * P:(vt + 1) * P],
                        rhs=qkvt[:, kt, 2 * D:3 * D],
                        start=(kt == 0), stop=(kt == D_T - 1))
                nc.scalar.activation(out=vpad[:, vt, :], in_=psv, func=AF.Copy)

            o_t = [medp.tile([64, T], BF16, tag=f"med{i}", name=f"o_t{i}")
                   for i in range(8)]
            for hp in range(4):
                for qp in range(4):
                    for hi in range(2):
                        h = 2 * hp + hi
                        base = hi * 64
                        av = psB.tile([64, 256], F32, tag="tp", name=f"av{h}_{qp}")
                        smx_h = []
                        for qh in range(2):
                            sc = psC.tile([P, 384], F32, tag="sc")
                            nc.tensor.matmul(
                                sc,
                                lhsT=q_all[base:base + 64, hp,
                                           qp * 256 + qh * P:qp * 256 + (qh + 1) * P],
                                rhs=kpad[base:base + 64, hp,
                                         qp * 256:qp * 256 + 384],
                                start=True, stop=True)
                            if qh == 0:
                                mi = 2 if qp == 0 else 0
                            else:
                                mi = 3 if qp == 3 else 1
                            smx = smxp.tile([P, 384], F32, tag="smx")
                            nc.vector.tensor_add(out=smx, in0=sc,
                                                 in1=masks_t[:, mi, :])
                            lsum = smp.tile([P, 1], F32, tag="lsum")
                            nc.scalar.activation(out=smx, in_=smx, func=AF.Exp,
                                                 accum_out=lsum)
                            rr = smp.tile([P, 1], F32, tag="rr")
                            nc.vector.reciprocal(out=rr, in_=lsum)
                            smxb = smxp.tile([P, 384], BF16, tag="smxb")
                            nc.vector.tensor_scalar(out=smxb, in0=smx, scalar1=rr,
                                                    scalar2=None, op0=OP.mult)
                            smx_h.append(smxb)
                        for sb in range(3):
                            tp2 = psB.tile([P, 256], BF16, tag="tp")
                            for qh in range(2):
                                nc.tensor.transpose(
                                    tp2[:, qh * P:(qh + 1) * P],
                                    smx_h[qh][:, sb * P:(sb + 1) * P],
                                    identR)
                            at = atp.tile([P, 256], BF16, tag="at")
                            nc.scalar.activation(out=at, in_=tp2, func=AF.Copy)
                            vt = qp * 2 + sb
                            nc.tensor.matmul(
                                av,
                                lhsT=vpad[:, vt, h * DH:(h + 1) * DH],
                                rhs=at,
                                start=(sb == 0), stop=(sb == 2))
                        nc.scalar.activation(
                            out=o_t[h][:, qp * 256:(qp + 1) * 256],
                            in_=av, func=AF.Identity,
                            bias=vb_t[:, h:h + 1], scale=1.0)
            # out projection + residual
            outwt = wbp.tile([64, 8, D], BF16, tag="wbig")
            nc.sync.dma_start(out=outwt,
                              in_=outw_d.rearrange("(a p) d -> p a d", p=64))
            for tc in range(TT_N):
                pso = psA.tile([P, D], F32, tag="mm")
                for h in range(8):
                    nc.tensor.matmul(
                        pso,
                        lhsT=o_t[h][:, tc * P:(tc + 1) * P],
                        rhs=outwt[:, h, :],
                        start=(h == 0), stop=(h == 7))
                nc.vector.tensor_add(out=y[:, tc, :], in0=y[:, tc, :], in1=pso)
                nc.gpsimd.tensor_add(out=y[:, tc, :], in0=y[:, tc, :], in1=outb_bc)

            # ================= conv module =================
            ln_t = layer_norm_t(False)
            pw1tt = wbp.tile([P, D_T, 2 * DC], BF16, tag="wbig")
            nc.sync.dma_start(out=pw1tt,
                              in_=pw1t_d.rearrange("(a p) d -> p a d", p=P))
            g = bigp.tile([P, DC_T, G_W], BF16, tag="big")
            zfill(g[:, :, 0:CPAD], DC_T, CPAD)
            zfill(g[:, :, CPAD + T:G_W], DC_T, CPAD)
            for ct in range(DC_T):
                for tn in range(2):
                    ps_a2 = psA.tile([P, D], F32, tag="mm")
                    for kt in range(D_T):
                        nc.tensor.matmul(
                            ps_a2,
                            lhsT=pw1tt[:, kt, DC + ct * P:DC + (ct + 1) * P],
                            rhs=ln_t[:, kt, PAD + tn * D:PAD + (tn + 1) * D],
                            start=(kt == 0), stop=(kt == D_T - 1))
                    sig = smp.tile([P, D], F32, tag="sig", bufs=2)
                    nc.scalar.activation(out=sig, in_=ps_a2, func=AF.Sigmoid,
                                         bias=ba2_t[:, ct:ct + 1], scale=1.0)
                    ps_a1 = psA.tile([P, D], F32, tag="mm")
                    for kt in range(D_T):
                        nc.tensor.matmul(
                            ps_a1,
                            lhsT=pw1tt[:, kt, ct * P:(ct + 1) * P],
                            rhs=ln_t[:, kt, PAD + tn * D:PAD + (tn + 1) * D],
                            start=(kt == 0), stop=(kt == D_T - 1))
                    nc.vector.scalar_tensor_tensor(
                        out=g[:, ct, CPAD + tn * D:CPAD + (tn + 1) * D],
                        in0=ps_a1, scalar=ba1_t[:, ct:ct + 1], in1=sig,
                        op0=OP.add, op1=OP.mult)
            # depthwise conv: per-tap per-channel MACs on Pool + DVE engines
            # (keeps the power-throttled PE free for real GEMMs)
            cv = [medp.tile([P, T], BF16, tag=f"med{i}", name=f"cv{i}") for i in range(DC_T)]
            for ct in range(DC_T):
                for tn in range(2):
                    accV = smp.tile([P, D], F32, tag="accV", bufs=2)
                    for j in range(KTAP):
                        gs = g[:, ct, j + tn * D:j + tn * D + D]
                        wj = dww_t[:, ct, j:j + 1]
                        if j == 0:
                            nc.vector.tensor_scalar(out=accV, in0=gs, scalar1=wj,
                                                    scalar2=dwb_t[:, ct:ct + 1],
                                                    op0=OP.mult, op1=OP.add)
                        elif j == KTAP - 1:
                            nc.vector.scalar_tensor_tensor(
                                out=cv[ct][:, tn * D:(tn + 1) * D], in0=gs,
                                scalar=wj, in1=accV, op0=OP.mult, op1=OP.add)
                        else:
                            nc.vector.scalar_tensor_tensor(out=accV, in0=gs,
                                                           scalar=wj, in1=accV,
                                                           op0=OP.mult, op1=OP.add)
            # GroupNorm(1 group over C,T) stats
            stats_pk = smp.tile([P, 16], F32, tag="stpk")
            for ct in range(DC_T):
                st = smp.tile([P, 2, 6], F32, tag="st26")
                nc.vector.bn_stats(out=st[:, 0, :], in_=cv[ct][:, 0:D])
                nc.vector.bn_stats(out=st[:, 1, :], in_=cv[ct][:, D:T])
                mv = smp.tile([P, 2], F32, tag="mv")
                nc.vector.bn_aggr(out=mv, in_=st)
                nc.vector.tensor_copy(out=stats_pk[:, ct:ct + 1], in_=mv[:, 0:1])
                nc.vector.scalar_tensor_tensor(
                    out=stats_pk[:, 8 + ct:9 + ct], in0=mv[:, 0:1],
                    scalar=mv[:, 0:1], in1=mv[:, 1:2], op0=OP.mult, op1=OP.add)
            red = smp.tile([P, 16], F32, tag="gred")
            nc.gpsimd.partition_all_reduce(red, stats_pk, channels=P,
                                           reduce_op=bass_isa.ReduceOp.add)
            sums = smp.tile([P, 2], F32, tag="sums")
            nc.vector.tensor_reduce(out=sums,
                                    in_=red.rearrange("p (a b) -> p a b", a=2),
                                    axis=AX.X, op=OP.add)
            mq = smp.tile([P, 2], F32, tag="mq")  # [mu, E[x^2]] on every partition
            nc.vector.tensor_scalar(out=mq, in0=sums, scalar1=1.0 / DC,
                                    scalar2=None, op0=OP.mult)
            # var = E[x^2] - mu^2
            var_t = smp.tile([P, 1], F32, tag="var")
            nc.vector.tensor_scalar(out=var_t, in0=mq[:, 0:1], scalar1=mq[:, 0:1],
                                    scalar2=-1.0, op0=OP.mult, op1=OP.mult)
            nc.vector.tensor_add(out=var_t, in0=var_t, in1=mq[:, 1:2])
            rstd = smp.tile([P, 1], F32, tag="rstd")
            nc.scalar.activation(out=rstd, in_=var_t, func=AF.Sqrt,
                                 bias=eps_t, scale=1.0)
            nc.vector.reciprocal(out=rstd, in_=rstd)
            # per-channel-tile scale/shift + SiLU, then pw2 + residual
            pw2tt = w2p.tile([P, DC_T, D], BF16, tag="w2res")
            nc.sync.dma_start(out=pw2tt,
                              in_=pw2t_d.rearrange("(a p) d -> p a d", p=P))
            for ct in range(DC_T):
                s_c = smp.tile([P, 1], F32, tag="s_c")
                nc.vector.tensor_scalar(out=s_c, in0=gnw_t[:, ct:ct + 1],
                                        scalar1=rstd, scalar2=None,
                                        op0=OP.mult)
                t_c = smp.tile([P, 1], F32, tag="t_c")
                nc.vector.tensor_scalar(out=t_c, in0=s_c, scalar1=mq[:, 0:1],
                                        scalar2=-1.0, op0=OP.mult, op1=OP.mult)
                nc.vector.tensor_add(out=t_c, in0=t_c, in1=gnb_t[:, ct:ct + 1])
                nc.scalar.activation(out=cv[ct], in_=cv[ct], func=AF.Silu,
                                     bias=t_c, scale=s_c)
            for tc in range(TT_N):
                psp = psA.tile([P, D], F32, tag="mm")
                for kt in range(DC_T):
                    nc.tensor.matmul(
                        psp,
                        lhsT=cv[kt][:, tc * P:(tc + 1) * P],
                        rhs=pw2tt[:, kt, :],
                        start=(kt == 0), stop=(kt == DC_T - 1))
                nc.vector.tensor_add(out=y[:, tc, :], in0=y[:, tc, :], in1=psp)
                nc.gpsimd.tensor_add(out=y[:, tc, :], in0=y[:, tc, :], in1=pw2b_bc)

            # ================= FFN2 =================
            ffn(w1f2_d, b1f2_t, w22_d, b2h2_bc)

            # ================= final LN + store =================
            for tt in range(TT_N):
                mv = smp.tile([P, 2], F32, tag="mv")
                st6 = smp.tile([P, 6], F32, tag="st6")
                nc.vector.bn_stats(out=st6, in_=y[:, tt, :])
                nc.vector.bn_aggr(out=mv, in_=st6)
                r_ = smp.tile([P, 1], F32, tag="r")
                nc.scalar.activation(out=r_, in_=mv[:, 1:2], func=AF.Sqrt,
                                     bias=eps_t, scale=1.0)
                nc.vector.reciprocal(out=r_, in_=r_)
                nmr = smp.tile([P, 1], F32, tag="nmr")
                nc.vector.tensor_scalar(out=nmr, in0=mv[:, 0:1], scalar1=r_,
                                        scalar2=-1.0, op0=OP.mult, op1=OP.mult)
                lnp = smp.tile([P, D], F32, tag="lnp", bufs=2)
                nc.vector.tensor_scalar(out=lnp, in0=y[:, tt, :], scalar1=r_,
                                        scalar2=nmr, op0=OP.mult, op1=OP.add)
                nc.vector.tensor_mul(out=lnp, in0=lnp, in1=flnw_bc)
                nc.vector.tensor_add(out=lnp, in0=lnp, in1=flnb_bc)
                nc.sync.dma_start(out=out_d[tt * P:(tt + 1) * P, :], in_=lnp)
    return nc


_NC_CACHE = {}


def _get_nc():
    if "nc" not in _NC_CACHE:
        nc = bacc.Bacc()
        _build(nc)
        nc.finalize()
        _NC_CACHE["nc"] = nc
    return _NC_CACHE["nc"]


def _prep_weights(inp):
    f = np.float32

    def a(x):
        return np.ascontiguousarray(np.asarray(x, dtype=f))

    def b(x):
        return np.ascontiguousarray(np.asarray(x, dtype=f).astype(NP_BF16))

    out = {}
    # FFN1: fold ln gamma/beta into w1/b1
    w1 = a(inp["ffn1_w1"]); lw = a(inp["ffn1_ln_w"]); lb = a(inp["ffn1_ln_b"])
    out["w1f"] = b(w1 * lw[:, None])
    b1 = a(inp["ffn1_b1"]) + lb @ w1
    out["b1f"] = a(b1.reshape(DF_T, P).T)
    out["w2"] = b(inp["ffn1_w2"])
    out["b2h"] = a(0.5 * a(inp["ffn1_b2"]))
    # attention
    qkvw = a(inp["qkv_w"]); alw = a(inp["attn_ln_w"]); alb = a(inp["attn_ln_b"])
    qkvf = qkvw * alw[:, None]
    qkvb = a(inp["qkv_b"]) + alb @ qkvw
    scale = np.float32(DH ** -0.5)
    qkvf[:, :D] *= scale
    out["qkvw"] = b(qkvf)
    out["qb"] = a((qkvb[:D] * scale).reshape(4, P).T)
    out["kb"] = a(qkvb[D:2 * D].reshape(4, P).T)
    out["vb"] = a(qkvb[2 * D:].reshape(8, 64).T)
    out["outw"] = b(inp["out_w"])
    out["outb"] = a(inp["out_b"])
    # conv module
    pw1 = a(inp["pw1_w"]); clw = a(inp["conv_ln_w"]); clb = a(inp["conv_ln_b"])
    out["pw1t"] = b((pw1 * clw[None, :]).T)
    pb = a(inp["pw1_b"]) + pw1 @ clb
    out["ba1"] = a(pb[:DC].reshape(DC_T, P).T)
    out["ba2"] = a(pb[DC:].reshape(DC_T, P).T)
    dw = a(inp["dw_w"]).reshape(DC, KTAP)
    out["dww"] = a(dw.reshape(DC_T, P, KTAP).transpose(1, 0, 2))
    out["dwb"] = a(a(inp["dw_b"]).reshape(DC_T, P).T)
    out["gnw"] = a(a(inp["gn_w"]).reshape(DC_T, P).T)
    out["gnb"] = a(a(inp["gn_b"]).reshape(DC_T, P).T)
    out["pw2t"] = b(a(inp["pw2_w"]).T)
    out["pw2b"] = a(inp["pw2_b"])
    # FFN2
    w12 = a(inp["ffn2_w1"]); lw2 = a(inp["ffn2_ln_w"]); lb2 = a(inp["ffn2_ln_b"])
    out["w1f2"] = b(w12 * lw2[:, None])
    b12 = a(inp["ffn2_b1"]) + lb2 @ w12
    out["b1f2"] = a(b12.reshape(DF_T, P).T)
    out["w22"] = b(inp["ffn2_w2"])
    out["b2h2"] = a(0.5 * a(inp["ffn2_b2"]))
    out["flnw"] = a(inp["final_ln_w"])
    out["flnb"] = a(inp["final_ln_b"])
    out["ident"] = np.eye(P, dtype=f).astype(NP_BF16)
    out["zeros"] = np.zeros(128, dtype=NP_BF16)
    # attention masks: [4, 128, 384]; additive
    masks = np.full((4, P, 384), NEG, dtype=f)
    i = np.arange(P)[:, None]
    p = np.arange(384)[None, :]
    w2_ = WIN // 2
    # qh0 interior: valid p in [i, i+64]
    masks[0][(p >= i) & (p <= i + 2 * w2_)] = 0.0
    # qh1 interior: valid p in [128+i, 192+i]
    masks[1][(p >= P + i) & (p <= P + i + 2 * w2_)] = 0.0
    # left edge (qp0,qh0): additionally p >= 32 (keys >= 0)
    masks[2][(p >= i) & (p <= i + 2 * w2_) & (p >= PAD)] = 0.0
    # right edge (qp3,qh1): additionally p < 288 (keys < 1024)
    masks[3][(p >= P + i) & (p <= P + i + 2 * w2_) & (p < 288)] = 0.0
    out["masks"] = a(masks.transpose(1, 0, 2))
    return out


def kernel(**inputs):
    x = np.asarray(inputs["x"], dtype=np.float32)
    assert x.shape == (B, T, D)
    weights = _prep_weights(inputs)
    nc = _get_nc()
    in_maps = []
    for i in range(N_CORES):
        m = dict(weights)
        m["x"] = np.ascontiguousarray(x[i])
        in_maps.append(m)
    res = run_bass_kernel_spmd(nc, in_maps, core_ids=list(range(N_CORES)))
    outs = [res.results[i]["y_out"] for i in range(N_CORES)]
    return np.stack(outs, axis=0).astype(np.float32)


if __name__ == "__main__":
    rng = np.random.default_rng(0)
    pass

